# revision 23
# baseline (speedup 1.0000x reference)
"""GCN layer (gather -> scale -> segment_sum -> leaky_relu) on 8 Trainium2 cores.

Sharding: output rows (nodes) are sharded across the 8 cores -- core c owns
rows [c*12500, (c+1)*12500). Each core's segment sums are then complete for
its own row range, so no collective is needed.

Per-core layout (built on the host):
  - rows are processed in 98 tiles of 128 rows;
  - edges are bucketed by (tile, col-bucket of 25000 source rows) and each
    bucket's edge list is tail-padded with idx=-1 to a uniform capacity
    (the dma_gather ucode trims trailing negative indices at zero cost);
  - embeds[col] rows are fetched with the custom dma_gather ucode
    (int16 indices relative to the 25000-row bucket base, wrapped in 16
    partitions and replicated): gathered row i of a call lands on
    partition i%128, chunk i//128. The four buckets of a tile issue on
    SWDGE queues 0..3, which run on different Q7 core pairs -- 4-way
    parallel descriptor generation (the bottleneck resource at ~8ns/desc).

Per 128-edge chunk the DVE builds S_val[e, r] = val[e] * (row_local[e]==r)
(r over the tile's 128 rows) with one chained tensor_scalar op, and the
TensorEngine accumulates psum[d, r] += E_gath[e, d]^T @ S_val[e, r].
PSUM is evacuated through leaky_relu = max(x, 0.5x) on the DVE into a
[d, rows] SBUF accumulator flushed to HBM every 7 tiles. The host
transposes/concatenates the per-core [128, 12544] outputs.
"""

import sys

sys.path.insert(0, "/opt/trn_rl_repo")

import numpy as np

N_NODES = 100000
D = 128
LEAKY_SLOPE = 0.5
N_CORES = 8
ROWS_PER_CORE = 12500
TILE_ROWS = 128
TILES = 98  # 98*128 = 12544 >= 12500
PAD_ROWS = TILES * TILE_ROWS
NB = 4  # column buckets == SWDGE queues
WB = 25000  # bucket width (int16-addressable)
OUT_FLUSH_TILES = 7
EBUF_BUFS = 4

_BUILD_CACHE = {}


def _build_bass(qb: int, repeat: int = 1, nqueues: int = NB,
                do_gather: bool = True, do_compute: bool = True):
    """qb = 128-edge chunks per (tile, bucket) segment.

    repeat > 1 wraps the whole tile loop in an on-device For_i that redoes
    the identical work `repeat` times -- used only for benchmarking (the
    per-call dispatch overhead through axon is ~100ms, far above the
    kernel's runtime, so timing uses wall(K)-wall(1) deltas)."""
    import contextlib

    import concourse.bacc as bacc
    import concourse.mybir as mybir
    import concourse.tile as tile

    f32 = mybir.dt.float32
    i32 = mybir.dt.int32
    i16 = mybir.dt.int16

    CAP = qb * 128  # capacity per (tile, bucket)
    NCHUNK = NB * qb  # chunks per tile
    IDX_COLS = NB * CAP // 16

    nc = bacc.Bacc("TRN2", target_bir_lowering=False, debug=False,
                   num_devices=N_CORES, num_swdge_queues=nqueues)

    emb = nc.dram_tensor("embeds", [N_NODES, D], f32, kind="ExternalInput")
    # last NB columns: per-bucket valid-index counts (ring-space accounting
    # in the dma_gather decode uses num_idxs_reg, which must equal the
    # number of non-negative indices -- see decode/dma_gather.hpp)
    meta32 = nc.dram_tensor("meta32", [TILES, 128, 2 * NCHUNK + NB], i32,
                            kind="ExternalInput")
    meta16 = nc.dram_tensor("meta16", [TILES, 128, IDX_COLS], i16,
                            kind="ExternalInput")
    iota = nc.dram_tensor("iota", [128, TILE_ROWS], f32,
                          kind="ExternalInput")
    out = nc.dram_tensor("out", [D, PAD_ROWS], f32, kind="ExternalOutput")

    with tile.TileContext(nc) as tc:
        with (
            tc.tile_pool(name="const", bufs=1) as const_pool,
            tc.tile_pool(name="meta", bufs=4) as meta_pool,
            tc.tile_pool(name="ebuf", bufs=EBUF_BUFS) as ebuf_pool,
            tc.tile_pool(name="sval", bufs=6) as sval_pool,
            tc.tile_pool(name="evac", bufs=4) as evac_pool,
            tc.tile_pool(name="acc", bufs=2) as acc_pool,
            tc.tile_pool(name="psum", bufs=6, space="PSUM") as psum_pool,
            tc.tile_pool(name="psiota", bufs=1, space="PSUM") as psiota_pool,
        ):
            iota_t = const_pool.tile([128, TILE_ROWS], f32)
            nc.sync.dma_start(out=iota_t[:], in_=iota.ap()[:])
            # keep the iota operand in PSUM: a PSUM-sourced tensor_scalar
            # avoids the DVE 2-port SBUF perf mode, whose port locking
            # starves the Q7 SWDGE descriptor rings (gather gen would
            # otherwise serialize against the per-chunk DVE ops)
            iota_ps = psiota_pool.tile([128, TILE_ROWS], f32)
            nc.vector.tensor_copy(out=iota_ps[:], in_=iota_t[:])

            cnt_regs = [
                next(iter(nc.alloc_registers(
                    f"gcnt{j}", engines=[mybir.EngineType.Pool]
                )))
                for j in range(NB)
            ]

            out_cols = OUT_FLUSH_TILES * TILE_ROWS  # 896
            acc_t = None

            if repeat > 1:
                loop_cm = tc.For_i(
                    0, repeat, 1,
                    hint_engines=(
                        mybir.EngineType.PE,
                        mybir.EngineType.DVE,
                        mybir.EngineType.Pool,
                        mybir.EngineType.SP,
                        mybir.EngineType.Activation,
                    ),
                )
            else:
                loop_cm = contextlib.nullcontext()
            with loop_cm:
                LOOKAHEAD = 2  # gathers run LOOKAHEAD tiles ahead of compute
                metas = {}
                ebufs = {}
                accs = {}

                def issue_meta(t):
                    if t >= TILES:
                        return
                    m32_t = meta_pool.tile([128, 2 * NCHUNK + NB], i32,
                                           tag="m32")
                    nc.sync.dma_start(out=m32_t[:], in_=meta32.ap()[t])
                    m16_t = meta_pool.tile([128, IDX_COLS], i16, tag="m16")
                    nc.sync.dma_start(out=m16_t[:], in_=meta16.ap()[t])
                    metas[t] = (m32_t, m16_t)

                def issue_gather(t):
                    if t >= TILES or not do_gather:
                        return
                    m32_t, m16_t = metas[t]
                    e_t = ebuf_pool.tile([128, NCHUNK, D], f32, tag="ebuf")
                    if t < EBUF_BUFS:
                        # first use of each physical slot: clear so that
                        # tail-pad chunks (never written by the gather)
                        # hold finite values (val=0 kills them in S_val)
                        nc.vector.memset(e_t[:], 0.0)
                    for j in range(NB):
                        c_col = 2 * NCHUNK + j
                        nc.reg_load(cnt_regs[j],
                                    m32_t[0:1, c_col:c_col + 1])
                        nc.gpsimd.dma_gather(
                            out_ap=e_t[:, j * qb:(j + 1) * qb, :],
                            in_ap=emb.ap()[j * WB:(j + 1) * WB, :],
                            idxs_ap=m16_t[:, j * (CAP // 16):
                                          (j + 1) * (CAP // 16)],
                            num_idxs=CAP,
                            num_idxs_reg=cnt_regs[j],
                            elem_size=D,
                            elem_step=D,
                            single_packet=False,
                            queue_num=j % nqueues,
                        )
                    ebufs[t] = e_t

                def issue_compute(t):
                    m32_t, m16_t = metas.pop(t)
                    if do_gather:
                        e_t = ebufs.pop(t)
                    else:
                        e_t = ebuf_pool.tile([128, NCHUNK, D], f32,
                                             tag="ebuf")
                        nc.vector.memset(e_t[:, 0, :], 0.0)
                    if t % OUT_FLUSH_TILES == 0:
                        acc_new = acc_pool.tile([128, out_cols], f32,
                                                tag="acc")
                        accs[0] = acc_new
                    acc_t = accs[0]
                    ps = psum_pool.tile([128, TILE_ROWS], f32, tag="ps")
                    for s in (range(NCHUNK) if do_compute else range(1)):
                        sv = sval_pool.tile([128, TILE_ROWS], f32, tag="sv")
                        nc.vector.tensor_scalar(
                            out=sv[:],
                            in0=iota_ps[:],
                            scalar1=m32_t[:, s:s + 1].bitcast(f32),
                            scalar2=m32_t[:, NCHUNK + s:NCHUNK + s + 1]
                            .bitcast(f32),
                            op0=mybir.AluOpType.is_equal,
                            op1=mybir.AluOpType.mult,
                        )
                        nc.tensor.matmul(
                            ps[:],
                            lhsT=e_t[:, s, :],
                            rhs=sv[:],
                            start=(s == 0),
                            stop=(s == NCHUNK - 1) or not do_compute,
                        )
                    col0 = (t % OUT_FLUSH_TILES) * TILE_ROWS
                    half_t = evac_pool.tile([128, TILE_ROWS], f32,
                                            tag="half")
                    nc.vector.tensor_scalar_mul(half_t[:], ps[:],
                                                LEAKY_SLOPE)
                    nc.vector.tensor_tensor(
                        out=acc_t[:, col0:col0 + TILE_ROWS],
                        in0=ps[:],
                        in1=half_t[:],
                        op=mybir.AluOpType.max,
                    )
                    if t % OUT_FLUSH_TILES == OUT_FLUSH_TILES - 1:
                        c0 = (t - (OUT_FLUSH_TILES - 1)) * TILE_ROWS
                        nc.sync.dma_start(
                            out=out.ap()[:, c0:c0 + out_cols],
                            in_=acc_t[:],
                        )

                for t in range(LOOKAHEAD):
                    issue_meta(t)
                    issue_gather(t)
                for t in range(TILES):
                    issue_meta(t + LOOKAHEAD)
                    issue_gather(t + LOOKAHEAD)
                    issue_compute(t)
    nc.compile()
    return nc


def _prep_inputs(edge_index, edge_vals, embeds):
    """Bucket edges into (core, tile, col-bucket) segments tail-padded to a
    uniform multiple-of-128 capacity; emit per-core meta arrays."""
    row = np.asarray(edge_index[0], dtype=np.int64)
    col = np.asarray(edge_index[1], dtype=np.int64)
    val = np.asarray(edge_vals, dtype=np.float32)
    embeds = np.ascontiguousarray(np.asarray(embeds, dtype=np.float32))

    core = row // ROWS_PER_CORE
    r_in_core = row - core * ROWS_PER_CORE
    t_idx = r_in_core // TILE_ROWS
    rl = (r_in_core % TILE_ROWS).astype(np.float32)
    j_idx = col // WB

    seg = (core * TILES + t_idx) * NB + j_idx
    nseg = N_CORES * TILES * NB

    order = np.argsort(seg, kind="stable")
    counts = np.bincount(seg, minlength=nseg)
    qb = max(2, int(-(-counts.max() // 128)))
    cap = qb * 128

    starts = np.cumsum(counts) - counts
    sseg = seg[order]
    pos = np.arange(row.size, dtype=np.int64) - starts[sseg]
    slots = sseg * cap + pos

    n_slots = nseg * cap
    idx16 = np.full(n_slots, -1, dtype=np.int16)  # tail pads trimmed by ucode
    rl_f = np.zeros(n_slots, dtype=np.float32)
    v_f = np.zeros(n_slots, dtype=np.float32)
    idx16[slots] = (col[order] - j_idx[order] * WB).astype(np.int16)
    rl_f[slots] = rl[order]
    v_f[slots] = val[order]

    # full shape: [cores, TILES, NB, qb, 128]
    shp = (N_CORES, TILES, NB, qb, 128)
    idx16 = idx16.reshape(shp)
    rl_f = rl_f.reshape(shp)
    v_f = v_f.reshape(shp)

    # meta32: [c, t, p, s] with s = j*qb + q
    def to_chunkcols(a):
        a = a.transpose(0, 1, 4, 2, 3)  # [c, t, p, j, q]
        return a.reshape(N_CORES, TILES, 128, NB * qb)

    rl_t = to_chunkcols(rl_f).view(np.int32)
    v_t = to_chunkcols(v_f).view(np.int32)
    cnt = counts.reshape(N_CORES, TILES, 1, NB).astype(np.int32)
    cnt = np.broadcast_to(cnt, (N_CORES, TILES, 128, NB))
    meta32 = np.ascontiguousarray(np.concatenate([rl_t, v_t, cnt], axis=3))

    # meta16: per call j, idx i = q*128 + p lives at (p%16, q*8 + p//16)
    a = idx16.reshape(N_CORES, TILES, NB, qb, 8, 16)  # p = p_hi*16 + p_lo
    a = a.transpose(0, 1, 5, 2, 3, 4)  # [c, t, p_lo, j, q, p_hi]
    a = a.reshape(N_CORES, TILES, 16, NB * qb * 8)
    meta16 = np.ascontiguousarray(np.tile(a, (1, 1, 8, 1)))

    iota = np.ascontiguousarray(
        np.tile(np.arange(TILE_ROWS, dtype=np.float32), (128, 1))
    )
    return embeds, meta32, meta16, iota, qb


def kernel(edge_index, edge_vals, embeds):
    from concourse.bass_utils import run_bass_kernel_spmd

    embeds_np, meta32, meta16, iota, qb = _prep_inputs(
        edge_index, edge_vals, embeds
    )

    if qb not in _BUILD_CACHE:
        _BUILD_CACHE[qb] = _build_bass(qb)
    nc = _BUILD_CACHE[qb]

    in_maps = [
        {"embeds": embeds_np, "meta32": meta32[c], "meta16": meta16[c],
         "iota": iota}
        for c in range(N_CORES)
    ]
    res = run_bass_kernel_spmd(nc, in_maps, core_ids=list(range(N_CORES)))

    out_full = np.empty((N_NODES, D), dtype=np.float32)
    for c in range(N_CORES):
        oc = res.results[c]["out"]  # [D, PAD_ROWS]
        out_full[c * ROWS_PER_CORE:(c + 1) * ROWS_PER_CORE] = \
            oc[:, :ROWS_PER_CORE].T
    return out_full


# revision 24
# speedup vs baseline: 1.0047x; 1.0047x over previous
"""GCN layer (gather -> scale -> segment_sum -> leaky_relu) on 8 Trainium2 cores.

Sharding: output rows (nodes) are sharded across the 8 cores -- core c owns
rows [c*12500, (c+1)*12500). Each core's segment sums are then complete for
its own row range, so no collective is needed.

Per-core layout (built on the host):
  - rows are processed in 98 tiles of 128 rows;
  - edges are bucketed by (tile, col-bucket of 25000 source rows) and each
    bucket's edge list is tail-padded with idx=-1 to a uniform capacity
    (the dma_gather ucode trims trailing negative indices at zero cost);
  - embeds[col] rows are fetched with the custom dma_gather ucode
    (int16 indices relative to the 25000-row bucket base, wrapped in 16
    partitions and replicated): gathered row i of a call lands on
    partition i%128, chunk i//128. The four buckets of a tile issue on
    SWDGE queues 0..3, which run on different Q7 core pairs -- 4-way
    parallel descriptor generation (the bottleneck resource at ~8ns/desc).

Per 128-edge chunk the DVE builds S_val[e, r] = val[e] * (row_local[e]==r)
(r over the tile's 128 rows) with one chained tensor_scalar op, and the
TensorEngine accumulates psum[d, r] += E_gath[e, d]^T @ S_val[e, r].
PSUM is evacuated through leaky_relu = max(x, 0.5x) on the DVE into a
[d, rows] SBUF accumulator flushed to HBM every 7 tiles. The host
transposes/concatenates the per-core [128, 12544] outputs.
"""

import sys

sys.path.insert(0, "/opt/trn_rl_repo")

import numpy as np

N_NODES = 100000
D = 128
LEAKY_SLOPE = 0.5
N_CORES = 8
ROWS_PER_CORE = 12500
TILE_ROWS = 128
TILES = 98  # 98*128 = 12544 >= 12500
PAD_ROWS = TILES * TILE_ROWS
NB = 4  # column buckets == SWDGE queues
WB = 25000  # bucket width (int16-addressable)
OUT_FLUSH_TILES = 7
EBUF_BUFS = 6

_BUILD_CACHE = {}


def _build_bass(qb: int, repeat: int = 1, nqueues: int = NB,
                do_gather: bool = True, do_compute: bool = True):
    """qb = 128-edge chunks per (tile, bucket) segment.

    repeat > 1 wraps the whole tile loop in an on-device For_i that redoes
    the identical work `repeat` times -- used only for benchmarking (the
    per-call dispatch overhead through axon is ~100ms, far above the
    kernel's runtime, so timing uses wall(K)-wall(1) deltas)."""
    import contextlib

    import concourse.bacc as bacc
    import concourse.mybir as mybir
    import concourse.tile as tile

    f32 = mybir.dt.float32
    i32 = mybir.dt.int32
    i16 = mybir.dt.int16

    CAP = qb * 128  # capacity per (tile, bucket)
    NCHUNK = NB * qb  # chunks per tile
    IDX_COLS = NB * CAP // 16

    nc = bacc.Bacc("TRN2", target_bir_lowering=False, debug=False,
                   num_devices=N_CORES, num_swdge_queues=nqueues)

    emb = nc.dram_tensor("embeds", [N_NODES, D], f32, kind="ExternalInput")
    # last NB columns: per-bucket valid-index counts (ring-space accounting
    # in the dma_gather decode uses num_idxs_reg, which must equal the
    # number of non-negative indices -- see decode/dma_gather.hpp)
    meta32 = nc.dram_tensor("meta32", [TILES, 128, 2 * NCHUNK + NB], i32,
                            kind="ExternalInput")
    meta16 = nc.dram_tensor("meta16", [TILES, 128, IDX_COLS], i16,
                            kind="ExternalInput")
    iota = nc.dram_tensor("iota", [128, TILE_ROWS], f32,
                          kind="ExternalInput")
    out = nc.dram_tensor("out", [D, PAD_ROWS], f32, kind="ExternalOutput")

    with tile.TileContext(nc) as tc:
        with (
            tc.tile_pool(name="const", bufs=1) as const_pool,
            tc.tile_pool(name="meta", bufs=4) as meta_pool,
            tc.tile_pool(name="ebuf", bufs=EBUF_BUFS) as ebuf_pool,
            tc.tile_pool(name="sval", bufs=6) as sval_pool,
            tc.tile_pool(name="evac", bufs=4) as evac_pool,
            tc.tile_pool(name="acc", bufs=2) as acc_pool,
            tc.tile_pool(name="psum", bufs=6, space="PSUM") as psum_pool,
            tc.tile_pool(name="psiota", bufs=1, space="PSUM") as psiota_pool,
        ):
            iota_t = const_pool.tile([128, TILE_ROWS], f32)
            nc.sync.dma_start(out=iota_t[:], in_=iota.ap()[:])
            # keep the iota operand in PSUM: a PSUM-sourced tensor_scalar
            # avoids the DVE 2-port SBUF perf mode, whose port locking
            # starves the Q7 SWDGE descriptor rings (gather gen would
            # otherwise serialize against the per-chunk DVE ops)
            iota_ps = psiota_pool.tile([128, TILE_ROWS], f32)
            nc.vector.tensor_copy(out=iota_ps[:], in_=iota_t[:])

            cnt_regs = [
                next(iter(nc.alloc_registers(
                    f"gcnt{j}", engines=[mybir.EngineType.Pool]
                )))
                for j in range(NB)
            ]

            out_cols = OUT_FLUSH_TILES * TILE_ROWS  # 896
            acc_t = None

            if repeat > 1:
                loop_cm = tc.For_i(
                    0, repeat, 1,
                    hint_engines=(
                        mybir.EngineType.PE,
                        mybir.EngineType.DVE,
                        mybir.EngineType.Pool,
                        mybir.EngineType.SP,
                        mybir.EngineType.Activation,
                    ),
                )
            else:
                loop_cm = contextlib.nullcontext()
            with loop_cm:
                LOOKAHEAD = 4  # gathers run LOOKAHEAD tiles ahead of compute
                metas = {}
                ebufs = {}
                accs = {}

                def issue_meta(t):
                    if t >= TILES:
                        return
                    m32_t = meta_pool.tile([128, 2 * NCHUNK + NB], i32,
                                           tag="m32")
                    nc.sync.dma_start(out=m32_t[:], in_=meta32.ap()[t])
                    m16_t = meta_pool.tile([128, IDX_COLS], i16, tag="m16")
                    nc.sync.dma_start(out=m16_t[:], in_=meta16.ap()[t])
                    metas[t] = (m32_t, m16_t)

                def issue_gather(t):
                    if t >= TILES or not do_gather:
                        return
                    m32_t, m16_t = metas[t]
                    e_t = ebuf_pool.tile([128, NCHUNK, D], f32, tag="ebuf")
                    if t < EBUF_BUFS:
                        # first use of each physical slot: clear so that
                        # tail-pad chunks (never written by the gather)
                        # hold finite values (val=0 kills them in S_val)
                        nc.vector.memset(e_t[:], 0.0)
                    for j in range(NB):
                        c_col = 2 * NCHUNK + j
                        nc.reg_load(cnt_regs[j],
                                    m32_t[0:1, c_col:c_col + 1])
                        nc.gpsimd.dma_gather(
                            out_ap=e_t[:, j * qb:(j + 1) * qb, :],
                            in_ap=emb.ap()[j * WB:(j + 1) * WB, :],
                            idxs_ap=m16_t[:, j * (CAP // 16):
                                          (j + 1) * (CAP // 16)],
                            num_idxs=CAP,
                            num_idxs_reg=cnt_regs[j],
                            elem_size=D,
                            elem_step=D,
                            single_packet=False,
                            queue_num=j % nqueues,
                        )
                    ebufs[t] = e_t

                def issue_compute(t):
                    m32_t, m16_t = metas.pop(t)
                    if do_gather:
                        e_t = ebufs.pop(t)
                    else:
                        e_t = ebuf_pool.tile([128, NCHUNK, D], f32,
                                             tag="ebuf")
                        nc.vector.memset(e_t[:, 0, :], 0.0)
                    if t % OUT_FLUSH_TILES == 0:
                        acc_new = acc_pool.tile([128, out_cols], f32,
                                                tag="acc")
                        accs[0] = acc_new
                    acc_t = accs[0]
                    ps = psum_pool.tile([128, TILE_ROWS], f32, tag="ps")
                    for s in (range(NCHUNK) if do_compute else range(1)):
                        sv = sval_pool.tile([128, TILE_ROWS], f32, tag="sv")
                        nc.vector.tensor_scalar(
                            out=sv[:],
                            in0=iota_ps[:],
                            scalar1=m32_t[:, s:s + 1].bitcast(f32),
                            scalar2=m32_t[:, NCHUNK + s:NCHUNK + s + 1]
                            .bitcast(f32),
                            op0=mybir.AluOpType.is_equal,
                            op1=mybir.AluOpType.mult,
                        )
                        nc.tensor.matmul(
                            ps[:],
                            lhsT=e_t[:, s, :],
                            rhs=sv[:],
                            start=(s == 0),
                            stop=(s == NCHUNK - 1) or not do_compute,
                        )
                    col0 = (t % OUT_FLUSH_TILES) * TILE_ROWS
                    half_t = evac_pool.tile([128, TILE_ROWS], f32,
                                            tag="half")
                    nc.vector.tensor_scalar_mul(half_t[:], ps[:],
                                                LEAKY_SLOPE)
                    nc.vector.tensor_tensor(
                        out=acc_t[:, col0:col0 + TILE_ROWS],
                        in0=ps[:],
                        in1=half_t[:],
                        op=mybir.AluOpType.max,
                    )
                    if t % OUT_FLUSH_TILES == OUT_FLUSH_TILES - 1:
                        c0 = (t - (OUT_FLUSH_TILES - 1)) * TILE_ROWS
                        nc.sync.dma_start(
                            out=out.ap()[:, c0:c0 + out_cols],
                            in_=acc_t[:],
                        )

                for t in range(LOOKAHEAD):
                    issue_meta(t)
                    issue_gather(t)
                for t in range(TILES):
                    issue_meta(t + LOOKAHEAD)
                    issue_gather(t + LOOKAHEAD)
                    issue_compute(t)
    nc.compile()
    return nc


def _prep_inputs(edge_index, edge_vals, embeds):
    """Bucket edges into (core, tile, col-bucket) segments tail-padded to a
    uniform multiple-of-128 capacity; emit per-core meta arrays."""
    row = np.asarray(edge_index[0], dtype=np.int64)
    col = np.asarray(edge_index[1], dtype=np.int64)
    val = np.asarray(edge_vals, dtype=np.float32)
    embeds = np.ascontiguousarray(np.asarray(embeds, dtype=np.float32))

    core = row // ROWS_PER_CORE
    r_in_core = row - core * ROWS_PER_CORE
    t_idx = r_in_core // TILE_ROWS
    rl = (r_in_core % TILE_ROWS).astype(np.float32)
    j_idx = col // WB

    seg = (core * TILES + t_idx) * NB + j_idx
    nseg = N_CORES * TILES * NB

    order = np.argsort(seg, kind="stable")
    counts = np.bincount(seg, minlength=nseg)
    qb = max(2, int(-(-counts.max() // 128)))
    cap = qb * 128

    starts = np.cumsum(counts) - counts
    sseg = seg[order]
    pos = np.arange(row.size, dtype=np.int64) - starts[sseg]
    slots = sseg * cap + pos

    n_slots = nseg * cap
    idx16 = np.full(n_slots, -1, dtype=np.int16)  # tail pads trimmed by ucode
    rl_f = np.zeros(n_slots, dtype=np.float32)
    v_f = np.zeros(n_slots, dtype=np.float32)
    idx16[slots] = (col[order] - j_idx[order] * WB).astype(np.int16)
    rl_f[slots] = rl[order]
    v_f[slots] = val[order]

    # full shape: [cores, TILES, NB, qb, 128]
    shp = (N_CORES, TILES, NB, qb, 128)
    idx16 = idx16.reshape(shp)
    rl_f = rl_f.reshape(shp)
    v_f = v_f.reshape(shp)

    # meta32: [c, t, p, s] with s = j*qb + q
    def to_chunkcols(a):
        a = a.transpose(0, 1, 4, 2, 3)  # [c, t, p, j, q]
        return a.reshape(N_CORES, TILES, 128, NB * qb)

    rl_t = to_chunkcols(rl_f).view(np.int32)
    v_t = to_chunkcols(v_f).view(np.int32)
    cnt = counts.reshape(N_CORES, TILES, 1, NB).astype(np.int32)
    cnt = np.broadcast_to(cnt, (N_CORES, TILES, 128, NB))
    meta32 = np.ascontiguousarray(np.concatenate([rl_t, v_t, cnt], axis=3))

    # meta16: per call j, idx i = q*128 + p lives at (p%16, q*8 + p//16)
    a = idx16.reshape(N_CORES, TILES, NB, qb, 8, 16)  # p = p_hi*16 + p_lo
    a = a.transpose(0, 1, 5, 2, 3, 4)  # [c, t, p_lo, j, q, p_hi]
    a = a.reshape(N_CORES, TILES, 16, NB * qb * 8)
    meta16 = np.ascontiguousarray(np.tile(a, (1, 1, 8, 1)))

    iota = np.ascontiguousarray(
        np.tile(np.arange(TILE_ROWS, dtype=np.float32), (128, 1))
    )
    return embeds, meta32, meta16, iota, qb


def kernel(edge_index, edge_vals, embeds):
    from concourse.bass_utils import run_bass_kernel_spmd

    embeds_np, meta32, meta16, iota, qb = _prep_inputs(
        edge_index, edge_vals, embeds
    )

    if qb not in _BUILD_CACHE:
        _BUILD_CACHE[qb] = _build_bass(qb)
    nc = _BUILD_CACHE[qb]

    in_maps = [
        {"embeds": embeds_np, "meta32": meta32[c], "meta16": meta16[c],
         "iota": iota}
        for c in range(N_CORES)
    ]
    res = run_bass_kernel_spmd(nc, in_maps, core_ids=list(range(N_CORES)))

    out_full = np.empty((N_NODES, D), dtype=np.float32)
    for c in range(N_CORES):
        oc = res.results[c]["out"]  # [D, PAD_ROWS]
        out_full[c * ROWS_PER_CORE:(c + 1) * ROWS_PER_CORE] = \
            oc[:, :ROWS_PER_CORE].T
    return out_full
